# revision 3
# baseline (speedup 1.0000x reference)
"""Multi-head attention layer on 8 Trainium2 NeuronCores.

Sharding: 8 cores = 2 (batch) x 4 (head groups of 4 heads).  Each core
computes its batch's attention for its 4 heads plus the partial output
projection (row-parallel); the host sums the 4 partials per batch.

Schedule (single software-pipelined stream per core):
  The softmax exp on ScalarE (ACTIVATE, ~142us total) is the hard
  bottleneck, so the program is ordered to keep ScalarE saturated from
  ~15us on.  Minimal ramp (kT m0, qT m0 lb0, v) is emitted first, then
  eight attention blocks (lb x head-pair) whose st-loops are pipelined
  (scores for st+1 issue while exp for st runs, ctx matmuls follow).
  All remaining projection work and the output projection are chopped
  into ~4-matmul "filler" units pulled into the PE stream between score
  groups, hiding them in the PE slack under the exp stream.

Per-core math (PE matmuls in bf16, fp32 PSUM accumulate):
  kT/qT = W_g @ x^T           (e=256 partitions, L free)
  v  = x @ Wv_g^T             (s x e layout, + per-head ones column)
  per head pair, l-block: scoresT = k_h q_h^T (row-tiled concurrent),
    E = exp(scale*scoresT) on ScalarE, ctx_aug += v_aug^T E (row 64 =
    rowsum), ctxT = ctx * (1/rowsum) (fast-approx recip + gpsimd bcast)
  out_partial = ctxT^T @ Wo_g^T, written bf16; host sums 4 partials.
"""

import ml_dtypes
import numpy as np

import concourse.bass as bass
import concourse.mybir as mybir
import concourse.tile as tile
from concourse import bacc
from concourse.bass_utils import run_bass_kernel_spmd

F32 = mybir.dt.float32
BF16 = mybir.dt.bfloat16
AF = mybir.ActivationFunctionType
ALU = mybir.AluOpType

P = 128
HD = 64  # head dim

D_MODEL = 1024
N_HEADS = 16
B = 2
L_FULL = 2048
N_CORES = 8
GROUPS = 4  # head groups (tensor parallel)
E_CORE = D_MODEL // GROUPS  # 256 output dims per core for q/k/v


def build_core_kernel(L=2048, D=1024, E=256, LB=512):
    """One core: x[b] attention over E//64 heads. Returns compiled Bacc."""
    assert L % LB == 0 and LB % P == 0 and D % P == 0 and E % HD == 0
    KT = D // P          # contraction tiles over d_model
    MT_E = E // P        # e tiles (q/k partition tiles)
    NLB = L // LB        # l blocks
    ST = L // P          # s tiles
    NH = E // HD         # heads on this core
    HP = NH // 2         # head pairs
    EA = NH * (HD + 1)   # v columns incl. per-head ones column
    SCALE = HD ** -0.5
    assert LB == 512 and MT_E == HP

    nc = bacc.Bacc("TRN2", target_bir_lowering=False, debug=False)

    xT = nc.declare_dram_parameter("xT", (D, L), BF16, isOutput=False)
    wqT = nc.declare_dram_parameter("wqT", (D, E), BF16, isOutput=False)
    wkT = nc.declare_dram_parameter("wkT", (D, E), BF16, isOutput=False)
    wvT = nc.declare_dram_parameter("wvT", (D, EA), BF16, isOutput=False)
    woT = nc.declare_dram_parameter("woT", (E, D), BF16, isOutput=False)
    bq = nc.declare_dram_parameter("bq", (MT_E, P), F32, isOutput=False)
    bk = nc.declare_dram_parameter("bk", (MT_E, P), F32, isOutput=False)
    bv = nc.declare_dram_parameter("bv", (1, EA), F32, isOutput=False)
    out = nc.declare_dram_parameter("out", (L, D), BF16, isOutput=True)

    with tile.TileContext(nc) as tc:
        with (
            tc.tile_pool(name="const", bufs=1) as const,
            tc.tile_pool(name="ps_sc", bufs=2, space="PSUM") as ps_sc,
            tc.tile_pool(name="ps_ctx", bufs=2, space="PSUM") as ps_ctx,
            tc.tile_pool(name="ps_proj", bufs=2, space="PSUM") as ps_proj,
            tc.tile_pool(name="ework", bufs=6) as ework,
            tc.tile_pool(name="small", bufs=8) as small,
            tc.tile_pool(name="outp", bufs=3) as outp,
        ):
            # ---- resident tensors ----
            xT_sb = const.tile([P, KT, L], BF16)
            wq_sb = const.tile([P, KT, E], BF16)
            wk_sb = const.tile([P, KT, E], BF16)
            wv_sb = const.tile([P, KT, EA], BF16)
            wo_sb = const.tile([P, MT_E, D], BF16)
            qT_sb = const.tile([P, MT_E, L], BF16)
            kT_sb = const.tile([P, MT_E, L], BF16)
            v_sb = const.tile([P, ST, NH, HD + 1], BF16)
            ctxT_sb = const.tile([P, MT_E, L], BF16)
            bq_sb = const.tile([P, MT_E], F32)
            bk_sb = const.tile([P, MT_E], F32)
            bv_row = const.tile([1, EA], F32)
            bv_bc = const.tile([P, EA], F32)
            act_warm = const.tile([1, 2], F32)

            # Warm the exp ACT table + gpsimd IRAM during the DMA shadow.
            nc.scalar.activation(act_warm[:, 1:2], act_warm[:, 0:1], AF.Exp)

            # ---- DMA prologue, earliest-needed first ----
            def dma_x(nb):
                for o in range(KT):
                    nc.sync.dma_start(
                        xT_sb[:, o, nb * LB:(nb + 1) * LB],
                        xT[o * P:(o + 1) * P, nb * LB:(nb + 1) * LB],
                    )

            for o in range(KT):
                nc.sync.dma_start(wk_sb[:, o, :], wkT[o * P:(o + 1) * P, :])
            dma_x(0)
            for o in range(KT):
                nc.sync.dma_start(wq_sb[:, o, :], wqT[o * P:(o + 1) * P, :])
            nc.sync.dma_start(bq_sb[:, :], bq.rearrange("o p -> p o"))
            nc.sync.dma_start(bk_sb[:, :], bk.rearrange("o p -> p o"))
            dma_x(1)
            for o in range(KT):
                nc.sync.dma_start(wv_sb[:, o, :], wvT[o * P:(o + 1) * P, :])
            nc.sync.dma_start(bv_row[:, :], bv[:, :])
            nc.gpsimd.partition_broadcast(bv_bc[:], bv_row[:])
            dma_x(2)
            dma_x(3)
            for o in range(MT_E):
                nc.sync.dma_start(wo_sb[:, o, :], woT[o * P:(o + 1) * P, :])

            # ---- unit emitters ----
            def kq_halves(w_sb, dst, b_sb, m, nb):
                """k/q projection of one 512-col l-block, as two 4-MM halves."""
                cell = {}

                def h1():
                    cell["ps"] = ps_proj.tile([P, 512], F32, tag="proj",
                                              name="proj")
                    for kk in range(4):
                        nc.tensor.matmul(
                            cell["ps"][:],
                            lhsT=w_sb[:, kk, m * P:(m + 1) * P],
                            rhs=xT_sb[:, kk, nb * 512:(nb + 1) * 512],
                            start=(kk == 0),
                            stop=False,
                        )

                def h2():
                    for kk in range(4, KT):
                        nc.tensor.matmul(
                            cell["ps"][:],
                            lhsT=w_sb[:, kk, m * P:(m + 1) * P],
                            rhs=xT_sb[:, kk, nb * 512:(nb + 1) * 512],
                            start=False,
                            stop=(kk == KT - 1),
                        )
                    nc.vector.tensor_scalar_add(
                        dst[:, m, nb * 512:(nb + 1) * 512], cell["ps"][:],
                        b_sb[:, m:m + 1],
                    )

                return [h1, h2]

            def v_unit(st):
                ps = ps_proj.tile([P, 512], F32, tag="proj", name="proj")
                for kk in range(KT):
                    nc.tensor.matmul(
                        ps[:, :EA],
                        lhsT=xT_sb[:, kk, st * P:(st + 1) * P],
                        rhs=wv_sb[:, kk, :],
                        start=(kk == 0),
                        stop=(kk == KT - 1),
                    )
                nc.vector.tensor_tensor(
                    v_sb[:, st, :, :],
                    ps[:, :EA].rearrange("p (h e) -> p h e", h=NH),
                    bv_bc[:].rearrange("p (h e) -> p h e", h=NH),
                    ALU.add,
                )

            def outproj_unit(lt, dc):
                def f():
                    ps = ps_proj.tile([P, 512], F32, tag="proj", name="proj")
                    for kk in range(MT_E):
                        nc.tensor.matmul(
                            ps[:],
                            lhsT=ctxT_sb[:, kk, lt * P:(lt + 1) * P],
                            rhs=wo_sb[:, kk, dc * 512:(dc + 1) * 512],
                            start=(kk == 0),
                            stop=(kk == MT_E - 1),
                        )
                    ot = outp.tile([P, 512], BF16, tag="ot")
                    nc.vector.tensor_copy(ot[:], ps[:])
                    nc.sync.dma_start(
                        out[lt * P:(lt + 1) * P, dc * 512:(dc + 1) * 512],
                        ot[:],
                    )

                return f

            # filler queue: (need_key, closure), FIFO
            fillers = []

            def pull(n=1):
                for _ in range(min(n, len(fillers))):
                    fillers.pop(0)[1]()

            def drain_needed(key):
                while any(u[0] == key for u in fillers):
                    fillers.pop(0)[1]()

            # ---- attention block: pipelined st loop ----
            def attn_block(lb, hp):
                ctx_tiles = [
                    ps_ctx.tile([HD + 1, LB], F32, tag="ctxps", name="ctxps")
                    for _ in range(2)
                ]

                def emit_sc(st):
                    sc = ps_sc.tile([P, 2 * LB], F32, tag="sc", name="sc")
                    for hh in range(2):
                        off = hh * HD
                        nc.tensor.matmul(
                            sc[:, hh * LB:(hh + 1) * LB],
                            lhsT=kT_sb[off:off + HD, hp, st * P:(st + 1) * P],
                            rhs=qT_sb[off:off + HD, hp, lb * LB:(lb + 1) * LB],
                            start=True,
                            stop=True,
                        )
                    e_t = ework.tile([P, 2 * LB], BF16, tag="etile",
                                     name="etile")
                    nc.scalar.activation(e_t[:], sc[:], AF.Exp, scale=SCALE)
                    return e_t

                def emit_ctx(st, e_t):
                    for hh in range(2):
                        h = 2 * hp + hh
                        nc.tensor.matmul(
                            ctx_tiles[hh][:],
                            lhsT=v_sb[:, st, h, :],
                            rhs=e_t[:, hh * LB:(hh + 1) * LB],
                            start=(st == 0),
                            stop=(st == ST - 1),
                        )

                e_prev = emit_sc(0)
                for st in range(1, ST):
                    e_cur = emit_sc(st)
                    emit_ctx(st - 1, e_prev)
                    e_prev = e_cur
                    if st % 2 == 1:
                        pull(1)
                emit_ctx(ST - 1, e_prev)

                for hh in range(2):
                    h = 2 * hp + hh
                    off = (h * HD) % P
                    recip = small.tile([1, LB], F32, tag="recip")
                    nc.vector.reciprocal(recip[:], ctx_tiles[hh][HD:HD + 1, :])
                    bcast = small.tile([HD, LB], F32, tag="bcast")
                    nc.gpsimd.partition_broadcast(bcast[:], recip[:])
                    nc.vector.tensor_tensor(
                        ctxT_sb[off:off + HD, hp, lb * LB:(lb + 1) * LB],
                        ctx_tiles[hh][:HD, :],
                        bcast[:],
                        ALU.mult,
                    )

            # ---- ramp: minimum projections for attn(0,0) ----
            for nb in range(NLB):
                for h in kq_halves(wk_sb, kT_sb, bk_sb, 0, nb):
                    h()
            for h in kq_halves(wq_sb, qT_sb, bq_sb, 0, 0):
                h()
            for st in range(ST):
                v_unit(st)

            for nb in range(NLB):
                for h in kq_halves(wk_sb, kT_sb, bk_sb, 1, nb):
                    fillers.append((("k", 1), h))
            for h in kq_halves(wq_sb, qT_sb, bq_sb, 1, 0):
                fillers.append((("q", 1, 0), h))

            # ---- main stream ----
            for lb in range(NLB):
                if lb + 1 < NLB:
                    for h in kq_halves(wq_sb, qT_sb, bq_sb, 0, lb + 1):
                        fillers.append((("q", 0, lb + 1), h))
                    for h in kq_halves(wq_sb, qT_sb, bq_sb, 1, lb + 1):
                        fillers.append((("q", 1, lb + 1), h))
                for hp in range(HP):
                    if hp == 1:
                        drain_needed(("k", 1))
                        drain_needed(("q", 1, lb))
                    else:
                        drain_needed(("q", 0, lb))
                    attn_block(lb, hp)
                for i in range(NLB):
                    lt = lb * NLB + i
                    for dc in range(D // 512):
                        fillers.append((None, outproj_unit(lt, dc)))
            while fillers:
                fillers.pop(0)[1]()
    nc.compile()
    return nc


def _augment_wv(wv_slice):
    """Interleave a zero column after each head's 64 value columns."""
    e, d = wv_slice.shape
    nh = e // HD
    aug = np.zeros((nh * (HD + 1), d), dtype=np.float32)
    for h in range(nh):
        aug[h * (HD + 1):h * (HD + 1) + HD] = wv_slice[h * HD:(h + 1) * HD]
    return aug


def _augment_bv(bv_slice):
    """bv with 1.0 in each head's ones-column slot."""
    e = bv_slice.shape[0]
    nh = e // HD
    aug = np.zeros(nh * (HD + 1), dtype=np.float32)
    for h in range(nh):
        aug[h * (HD + 1):h * (HD + 1) + HD] = bv_slice[h * HD:(h + 1) * HD]
        aug[h * (HD + 1) + HD] = 1.0
    return aug


def _core_in_map(core, x, Wq, bq, Wk, bk, Wv, bv, Wo):
    b = core // GROUPS
    g = core % GROUPS
    sl = slice(g * E_CORE, (g + 1) * E_CORE)
    bf = ml_dtypes.bfloat16
    return {
        "xT": np.ascontiguousarray(x[b].T.astype(bf)),
        "wqT": np.ascontiguousarray(Wq[sl, :].T.astype(bf)),
        "wkT": np.ascontiguousarray(Wk[sl, :].T.astype(bf)),
        "wvT": np.ascontiguousarray(_augment_wv(Wv[sl, :]).T.astype(bf)),
        "woT": np.ascontiguousarray(Wo[:, sl].T.astype(bf)),
        "bq": np.ascontiguousarray(bq[sl].reshape(-1, 128)),
        "bk": np.ascontiguousarray(bk[sl].reshape(-1, 128)),
        "bv": np.ascontiguousarray(_augment_bv(bv[sl]).reshape(1, -1)),
    }


_NC_CACHE = {}


def _get_kernel(L, D, E):
    key = (L, D, E)
    if key not in _NC_CACHE:
        _NC_CACHE[key] = build_core_kernel(L=L, D=D, E=E)
    return _NC_CACHE[key]


LAST_RESULT = None


def kernel(x, Wq, bq, Wk, bk, Wv, bv, Wo, bo, trace=False, tmpdir=None):
    x = np.asarray(x, dtype=np.float32)
    Wq = np.asarray(Wq, dtype=np.float32)
    Wk = np.asarray(Wk, dtype=np.float32)
    Wv = np.asarray(Wv, dtype=np.float32)
    Wo = np.asarray(Wo, dtype=np.float32)
    bq = np.asarray(bq, dtype=np.float32)
    bk = np.asarray(bk, dtype=np.float32)
    bv = np.asarray(bv, dtype=np.float32)
    bo = np.asarray(bo, dtype=np.float32)

    Bx, L, D = x.shape
    nc = _get_kernel(L, D, E_CORE)

    in_maps = [
        _core_in_map(core, x, Wq, bq, Wk, bk, Wv, bv, Wo)
        for core in range(N_CORES)
    ]

    global LAST_RESULT
    LAST_RESULT = run_bass_kernel_spmd(
        nc, in_maps, core_ids=list(range(N_CORES)), trace=trace, tmpdir=tmpdir,
    )
    outs = [
        np.asarray(LAST_RESULT.results[c]["out"]).astype(np.float32)
        for c in range(N_CORES)
    ]
    full = np.stack(
        [sum(outs[b * GROUPS:(b + 1) * GROUPS]) for b in range(Bx)], axis=0
    )
    return (full + bo).astype(np.float32)


# revision 15
# speedup vs baseline: 1.3701x; 1.3701x over previous
"""Multi-head attention layer on 8 Trainium2 NeuronCores.

Sharding: 8 cores = 2 (batch) x 4 (head groups of 4 heads).  Each core
computes its batch's attention for its 4 heads plus the partial output
projection (row-parallel); the host sums the 4 partials per batch.

Schedule (single software-pipelined stream per core):
  The softmax exp on ScalarE (ACTIVATE, ~142us total over 128 calls) is
  the hard bottleneck, so the program keeps ScalarE saturated from ~12us
  on.  Attention blocks run hp=0 for all four l-blocks first, then hp=1,
  with the first block's st-loop carrying the remaining k/v projections
  as just-in-time inserts.  All other projection work and the output
  projection are chopped into small "filler" units pulled into the PE
  stream between score groups.  Input DMA issues are spread across the
  sync/gpsimd/scalar queues so descriptor issue rate doesn't gate the
  ramp.  The softmax normalisation copies ctx out of PSUM immediately
  (freeing the accumulation banks for the next block) and batches the
  two per-head reciprocals into one DVE instruction.

Per-core math (PE matmuls in bf16, fp32 PSUM accumulate):
  kT/qT = W_g @ x^T           (e=256 partitions, L free)
  v  = x @ Wv_g^T             (s x e layout, + per-head ones column)
  per head pair, l-block: scoresT = k_h q_h^T (row-tiled concurrent),
    E = exp(scale*scoresT) on ScalarE, ctx_aug += v_aug^T E (row 64 =
    rowsum), ctxT = ctx * (1/rowsum)
  out_partial = ctxT^T @ Wo_g^T, written bf16; host sums 4 partials.
"""

import ml_dtypes
import numpy as np

import concourse.bass as bass
import concourse.mybir as mybir
import concourse.tile as tile
from concourse import bacc
from concourse.bass_utils import run_bass_kernel_spmd

F32 = mybir.dt.float32
BF16 = mybir.dt.bfloat16
AF = mybir.ActivationFunctionType
ALU = mybir.AluOpType

P = 128
HD = 64  # head dim

D_MODEL = 1024
N_HEADS = 16
B = 2
L_FULL = 2048
N_CORES = 8
GROUPS = 4  # head groups (tensor parallel)
E_CORE = D_MODEL // GROUPS  # 256 output dims per core for q/k/v


def build_core_kernel(L=2048, D=1024, E=256, LB=512):
    """One core: x[b] attention over E//64 heads. Returns compiled Bacc."""
    assert L % LB == 0 and LB % P == 0 and D % P == 0 and E % HD == 0
    KT = D // P          # contraction tiles over d_model
    MT_E = E // P        # e tiles (q/k partition tiles)
    NLB = L // LB        # l blocks
    ST = L // P          # s tiles
    NH = E // HD         # heads on this core
    HP = NH // 2         # head pairs
    EA = NH * (HD + 1)   # v columns incl. per-head ones column
    SCALE = HD ** -0.5
    assert LB == 512 and MT_E == HP and NLB == 4 and ST == 16

    nc = bacc.Bacc("TRN2", target_bir_lowering=False, debug=False)

    xT = nc.declare_dram_parameter("xT", (D, L), BF16, isOutput=False)
    wqT = nc.declare_dram_parameter("wqT", (D, E), BF16, isOutput=False)
    wkT = nc.declare_dram_parameter("wkT", (D, E), BF16, isOutput=False)
    wvT = nc.declare_dram_parameter("wvT", (D, EA), BF16, isOutput=False)
    woT = nc.declare_dram_parameter("woT", (E, D), BF16, isOutput=False)
    bq = nc.declare_dram_parameter("bq", (MT_E, P), F32, isOutput=False)
    bk = nc.declare_dram_parameter("bk", (MT_E, P), F32, isOutput=False)
    bv = nc.declare_dram_parameter("bv", (P, EA), F32, isOutput=False)
    out = nc.declare_dram_parameter("out", (L, D), BF16, isOutput=True)

    with tile.TileContext(nc) as tc:
        with (
            tc.tile_pool(name="const", bufs=1) as const,
            tc.tile_pool(name="ps_sc", bufs=2, space="PSUM") as ps_sc,
            tc.tile_pool(name="ps_ctx", bufs=2, space="PSUM") as ps_ctx,
            tc.tile_pool(name="ps_proj", bufs=2, space="PSUM") as ps_proj,
            tc.tile_pool(name="ework", bufs=6) as ework,
            tc.tile_pool(name="small", bufs=6) as small,
            tc.tile_pool(name="outp", bufs=3) as outp,
        ):
            # ---- resident tensors ----
            xT_sb = const.tile([P, KT, L], BF16)
            wq_sb = const.tile([P, KT, E], BF16)
            wk_sb = const.tile([P, KT, E], BF16)
            wv_sb = const.tile([P, KT, EA], BF16)
            wo_sb = const.tile([P, MT_E, D], BF16)
            qT_sb = const.tile([P, MT_E, L], BF16)
            kT_sb = const.tile([P, MT_E, L], BF16)
            v_sb = const.tile([P, ST, NH, HD + 1], BF16)
            ctxT_sb = const.tile([P, MT_E, L], BF16)
            bq_sb = const.tile([P, MT_E], F32)
            bk_sb = const.tile([P, MT_E], F32)
            bv_bc = const.tile([P, EA], F32)
            act_warm = const.tile([1, 2], F32)

            # Warm the exp ACT table during the DMA shadow.
            nc.scalar.activation(act_warm[:, 1:2], act_warm[:, 0:1], AF.Exp)

            # ---- DMA prologue: spread issues over 4 engine queues ----
            def dma_x(eng, nb):
                for o in range(KT):
                    eng.dma_start(
                        xT_sb[:, o, nb * LB:(nb + 1) * LB],
                        xT[o * P:(o + 1) * P, nb * LB:(nb + 1) * LB],
                    )

            def dma_w(eng, dst, src):
                for o in range(KT):
                    eng.dma_start(dst[:, o, :], src[o * P:(o + 1) * P, :])

            # sync queue: wk first (kT m0 gate), bv (v_0 gate), x n1/n3
            dma_w(nc.sync, wk_sb, wkT)
            nc.sync.dma_start(bv_bc[:, :], bv[:, :])
            nc.sync.dma_start(bq_sb[:, :], bq.rearrange("o p -> p o"))
            nc.sync.dma_start(bk_sb[:, :], bk.rearrange("o p -> p o"))
            dma_x(nc.sync, 1)
            dma_x(nc.sync, 3)
            # gpsimd queue: x n0 (kT m0 + v_0 gate), x n2, wo
            dma_x(nc.gpsimd, 0)
            dma_x(nc.gpsimd, 2)
            for o in range(MT_E):
                nc.gpsimd.dma_start(wo_sb[:, o, :], woT[o * P:(o + 1) * P, :])
            # scalar queue: wq then wv (after the table-warm activation)
            dma_w(nc.scalar, wq_sb, wqT)
            dma_w(nc.scalar, wv_sb, wvT)

            # ---- unit emitters ----
            def kq_halves(w_sb, dst, b_sb, m, nb):
                """k/q projection of one 512-col l-block, as two 4-MM halves."""
                cell = {}

                def h1():
                    cell["ps"] = ps_proj.tile([P, 512], F32, tag="proj",
                                              name="proj")
                    for kk in range(4):
                        nc.tensor.matmul(
                            cell["ps"][:],
                            lhsT=w_sb[:, kk, m * P:(m + 1) * P],
                            rhs=xT_sb[:, kk, nb * 512:(nb + 1) * 512],
                            start=(kk == 0),
                            stop=False,
                        )

                def h2():
                    for kk in range(4, KT):
                        nc.tensor.matmul(
                            cell["ps"][:],
                            lhsT=w_sb[:, kk, m * P:(m + 1) * P],
                            rhs=xT_sb[:, kk, nb * 512:(nb + 1) * 512],
                            start=False,
                            stop=(kk == KT - 1),
                        )
                    nc.vector.tensor_scalar_add(
                        dst[:, m, nb * 512:(nb + 1) * 512], cell["ps"][:],
                        b_sb[:, m:m + 1],
                    )

                return [h1, h2]

            def v_unit(st):
                def f():
                    ps = ps_proj.tile([P, 512], F32, tag="proj", name="proj")
                    for kk in range(KT):
                        nc.tensor.matmul(
                            ps[:, :EA],
                            lhsT=xT_sb[:, kk, st * P:(st + 1) * P],
                            rhs=wv_sb[:, kk, :],
                            start=(kk == 0),
                            stop=(kk == KT - 1),
                        )
                    nc.vector.tensor_tensor(
                        v_sb[:, st, :, :],
                        ps[:, :EA].rearrange("p (h e) -> p h e", h=NH),
                        bv_bc[:].rearrange("p (h e) -> p h e", h=NH),
                        ALU.add,
                    )

                return f

            def outproj_unit(lt, dc):
                def f():
                    ps = ps_proj.tile([P, 512], F32, tag="proj", name="proj")
                    for kk in range(MT_E):
                        nc.tensor.matmul(
                            ps[:],
                            lhsT=ctxT_sb[:, kk, lt * P:(lt + 1) * P],
                            rhs=wo_sb[:, kk, dc * 512:(dc + 1) * 512],
                            start=(kk == 0),
                            stop=(kk == MT_E - 1),
                        )
                    ot = outp.tile([P, 512], BF16, tag="ot")
                    nc.vector.tensor_copy(ot[:], ps[:])
                    nc.sync.dma_start(
                        out[lt * P:(lt + 1) * P, dc * 512:(dc + 1) * 512],
                        ot[:],
                    )

                return f

            # filler queue: (need_key, closure), FIFO
            fillers = []

            def pull(n=1):
                for _ in range(min(n, len(fillers))):
                    fillers.pop(0)[1]()

            def drain_needed(key):
                while any(u[0] == key for u in fillers):
                    fillers.pop(0)[1]()

            # ---- attention block: pipelined st loop ----
            def attn_block(lb, hp, inserts=None):
                ctx_tiles = [
                    ps_ctx.tile([HD + 1, LB], F32, tag="ctxps", name="ctxps")
                    for _ in range(2)
                ]

                def emit_sc(st):
                    sc = ps_sc.tile([P, 2 * LB], F32, tag="sc", name="sc")
                    for hh in range(2):
                        off = hh * HD
                        nc.tensor.matmul(
                            sc[:, hh * LB:(hh + 1) * LB],
                            lhsT=kT_sb[off:off + HD, hp, st * P:(st + 1) * P],
                            rhs=qT_sb[off:off + HD, hp, lb * LB:(lb + 1) * LB],
                            start=True,
                            stop=True,
                        )
                    e_t = ework.tile([P, 2 * LB], BF16, tag="etile",
                                     name="etile")
                    nc.scalar.activation(e_t[:], sc[:], AF.Exp, scale=SCALE)
                    return e_t

                def emit_ctx(st, e_t):
                    for hh in range(2):
                        h = 2 * hp + hh
                        nc.tensor.matmul(
                            ctx_tiles[hh][:],
                            lhsT=v_sb[:, st, h, :],
                            rhs=e_t[:, hh * LB:(hh + 1) * LB],
                            start=(st == 0),
                            stop=(st == ST - 1),
                        )

                def filler_slot(st):
                    if inserts is not None:
                        for f in inserts.get(st, ()):
                            f()
                    else:
                        pull(1)

                e_prev = emit_sc(0)
                for st in range(1, ST):
                    e_cur = emit_sc(st)
                    filler_slot(st)
                    emit_ctx(st - 1, e_prev)
                    e_prev = e_cur
                filler_slot(ST)
                emit_ctx(ST - 1, e_prev)

                # normalisation: copy ctx out of PSUM at once (frees the
                # accumulation banks), one batched reciprocal, then scale.
                cc = [small.tile([HD, LB], F32, tag="cc", name="cc")
                      for _ in range(2)]
                rs = small.tile([HD, LB], F32, tag="rs", name="rs")
                nc.vector.memset(rs[:33, :], 1.0)
                for hh in range(2):
                    nc.vector.tensor_copy(cc[hh][:], ctx_tiles[hh][:HD, :])
                    nc.vector.tensor_copy(rs[32 * hh:32 * hh + 1, :],
                                          ctx_tiles[hh][HD:HD + 1, :])
                rp = small.tile([HD, LB], F32, tag="rp", name="rp")
                nc.vector.reciprocal(rp[:33, :], rs[:33, :])
                rp1 = small.tile([1, LB], F32, tag="rp1", name="rp1")
                nc.vector.tensor_copy(rp1[:], rp[32:33, :])
                for hh in range(2):
                    h = 2 * hp + hh
                    off = (h * HD) % P
                    bcast = small.tile([HD, LB], F32, tag="bcast",
                                       name="bcast")
                    nc.gpsimd.partition_broadcast(
                        bcast[:], rp[0:1, :] if hh == 0 else rp1[:])
                    nc.vector.tensor_tensor(
                        ctxT_sb[off:off + HD, hp, lb * LB:(lb + 1) * LB],
                        cc[hh][:],
                        bcast[:],
                        ALU.mult,
                    )

            # ---- ramp: bare minimum before the first score matmul ----
            for h in kq_halves(wk_sb, kT_sb, bk_sb, 0, 0):
                h()
            for h in kq_halves(wq_sb, qT_sb, bq_sb, 0, 0):
                h()

            # first block carries the k/v projections just-in-time:
            # v_{st-1} lands in slot st (just before ctx_{st-1}).
            kn = {nb: kq_halves(wk_sb, kT_sb, bk_sb, 0, nb) for nb in (1, 2, 3)}
            q1 = kq_halves(wq_sb, qT_sb, bq_sb, 0, 1)
            inserts0 = {
                1: [v_unit(0), kn[1][0]],
                2: [v_unit(1), kn[1][1]],
                3: [v_unit(2)],
                4: [v_unit(3), kn[2][0]],
                5: [v_unit(4), kn[2][1]],
                6: [v_unit(5)],
                7: [v_unit(6), kn[3][0]],
                8: [v_unit(7), kn[3][1]],
                9: [v_unit(8)],
                10: [v_unit(9), q1[0]],
                11: [v_unit(10), q1[1]],
                12: [v_unit(11)],
                13: [v_unit(12), v_unit(13)],
                14: [v_unit(14)],
                15: [v_unit(15)],
            }

            # filler queue, in global dependency order
            for h in kq_halves(wq_sb, qT_sb, bq_sb, 0, 2):
                fillers.append((("q", 0, 2), h))
            for nb in range(NLB):
                for h in kq_halves(wk_sb, kT_sb, bk_sb, 1, nb):
                    fillers.append((("k", 1), h))
            for h in kq_halves(wq_sb, qT_sb, bq_sb, 0, 3):
                fillers.append((("q", 0, 3), h))
            for lb in range(NLB):
                for h in kq_halves(wq_sb, qT_sb, bq_sb, 1, lb):
                    fillers.append((("q", 1, lb), h))

            # ---- main stream: hp=0 blocks, then hp=1 blocks ----
            attn_block(0, 0, inserts=inserts0)
            for lb in range(1, NLB):
                drain_needed(("q", 0, lb))
                attn_block(lb, 0)
            for lb in range(NLB):
                drain_needed(("k", 1))
                drain_needed(("q", 1, lb))
                attn_block(lb, 1)
                for i in range(NLB):
                    lt = lb * NLB + i
                    for dc in range(D // 512):
                        fillers.append((None, outproj_unit(lt, dc)))
            while fillers:
                fillers.pop(0)[1]()
    nc.compile()
    return nc


def _augment_wv(wv_slice):
    """Interleave a zero column after each head's 64 value columns."""
    e, d = wv_slice.shape
    nh = e // HD
    aug = np.zeros((nh * (HD + 1), d), dtype=np.float32)
    for h in range(nh):
        aug[h * (HD + 1):h * (HD + 1) + HD] = wv_slice[h * HD:(h + 1) * HD]
    return aug


def _augment_bv(bv_slice):
    """bv with 1.0 in each head's ones-column slot."""
    e = bv_slice.shape[0]
    nh = e // HD
    aug = np.zeros(nh * (HD + 1), dtype=np.float32)
    for h in range(nh):
        aug[h * (HD + 1):h * (HD + 1) + HD] = bv_slice[h * HD:(h + 1) * HD]
        aug[h * (HD + 1) + HD] = 1.0
    return aug


def _core_in_map(core, x, Wq, bq, Wk, bk, Wv, bv, Wo):
    b = core // GROUPS
    g = core % GROUPS
    sl = slice(g * E_CORE, (g + 1) * E_CORE)
    bf = ml_dtypes.bfloat16
    return {
        "xT": np.ascontiguousarray(x[b].T.astype(bf)),
        "wqT": np.ascontiguousarray(Wq[sl, :].T.astype(bf)),
        "wkT": np.ascontiguousarray(Wk[sl, :].T.astype(bf)),
        "wvT": np.ascontiguousarray(_augment_wv(Wv[sl, :]).T.astype(bf)),
        "woT": np.ascontiguousarray(Wo[:, sl].T.astype(bf)),
        "bq": np.ascontiguousarray(bq[sl].reshape(-1, 128)),
        "bk": np.ascontiguousarray(bk[sl].reshape(-1, 128)),
        "bv": np.ascontiguousarray(
            np.tile(_augment_bv(bv[sl]).reshape(1, -1), (128, 1))),
    }


_NC_CACHE = {}


def _get_kernel(L, D, E):
    key = (L, D, E)
    if key not in _NC_CACHE:
        _NC_CACHE[key] = build_core_kernel(L=L, D=D, E=E)
    return _NC_CACHE[key]


LAST_RESULT = None


def kernel(x, Wq, bq, Wk, bk, Wv, bv, Wo, bo, trace=False, tmpdir=None):
    x = np.asarray(x, dtype=np.float32)
    Wq = np.asarray(Wq, dtype=np.float32)
    Wk = np.asarray(Wk, dtype=np.float32)
    Wv = np.asarray(Wv, dtype=np.float32)
    Wo = np.asarray(Wo, dtype=np.float32)
    bq = np.asarray(bq, dtype=np.float32)
    bk = np.asarray(bk, dtype=np.float32)
    bv = np.asarray(bv, dtype=np.float32)
    bo = np.asarray(bo, dtype=np.float32)

    Bx, L, D = x.shape
    nc = _get_kernel(L, D, E_CORE)

    in_maps = [
        _core_in_map(core, x, Wq, bq, Wk, bk, Wv, bv, Wo)
        for core in range(N_CORES)
    ]

    global LAST_RESULT
    LAST_RESULT = run_bass_kernel_spmd(
        nc, in_maps, core_ids=list(range(N_CORES)), trace=trace, tmpdir=tmpdir,
    )
    outs = [
        np.asarray(LAST_RESULT.results[c]["out"]).astype(np.float32)
        for c in range(N_CORES)
    ]
    full = np.stack(
        [sum(outs[b * GROUPS:(b + 1) * GROUPS]) for b in range(Bx)], axis=0
    )
    return (full + bo).astype(np.float32)


# revision 20
# speedup vs baseline: 1.3967x; 1.0195x over previous
"""Multi-head attention layer on 8 Trainium2 NeuronCores.

Sharding: 8 cores = 2 (batch) x 4 (head groups of 4 heads).  Each core
computes its batch's attention for its 4 heads plus the partial output
projection (row-parallel); the host sums the 4 partials per batch.

Schedule (single software-pipelined stream per core):
  The softmax exp on ScalarE (ACTIVATE, ~142us total over 128 calls) is
  the hard bottleneck, so the program keeps ScalarE saturated from ~12us
  on.  Attention blocks run hp=0 for all four l-blocks first, then hp=1,
  with the first block's st-loop carrying the remaining k/v projections
  as just-in-time inserts.  All other projection work and the output
  projection are chopped into small "filler" units pulled into the PE
  stream between score groups.  Input DMA issues are spread across the
  sync/gpsimd/scalar queues so descriptor issue rate doesn't gate the
  ramp.  The softmax normalisation copies ctx out of PSUM immediately
  (freeing the accumulation banks for the next block) and batches the
  two per-head reciprocals into one DVE instruction.

Per-core math (PE matmuls in bf16, fp32 PSUM accumulate):
  kT/qT = W_g @ x^T           (e=256 partitions, L free)
  v  = x @ Wv_g^T             (s x e layout, + per-head ones column)
  per head pair, l-block: scoresT = k_h q_h^T (row-tiled concurrent),
    E = exp(scale*scoresT) on ScalarE, ctx_aug += v_aug^T E (row 64 =
    rowsum), ctxT = ctx * (1/rowsum)
  out_partial = ctxT^T @ Wo_g^T, written bf16; host sums 4 partials.
"""

import ml_dtypes
import numpy as np

import concourse.bass as bass
import concourse.mybir as mybir
import concourse.tile as tile
from concourse import bacc
from concourse.bass_utils import run_bass_kernel_spmd

F32 = mybir.dt.float32
BF16 = mybir.dt.bfloat16
AF = mybir.ActivationFunctionType
ALU = mybir.AluOpType

P = 128
HD = 64  # head dim

D_MODEL = 1024
N_HEADS = 16
B = 2
L_FULL = 2048
N_CORES = 8
GROUPS = 4  # head groups (tensor parallel)
E_CORE = D_MODEL // GROUPS  # 256 output dims per core for q/k/v


def build_core_kernel(L=2048, D=1024, E=256, LB=512):
    """One core: x[b] attention over E//64 heads. Returns compiled Bacc."""
    assert L % LB == 0 and LB % P == 0 and D % P == 0 and E % HD == 0
    KT = D // P          # contraction tiles over d_model
    MT_E = E // P        # e tiles (q/k partition tiles)
    NLB = L // LB        # l blocks
    ST = L // P          # s tiles
    NH = E // HD         # heads on this core
    HP = NH // 2         # head pairs
    EA = NH * (HD + 1)   # v columns incl. per-head ones column
    SCALE = HD ** -0.5
    assert LB == 512 and MT_E == HP and NLB == 4 and ST == 16

    nc = bacc.Bacc("TRN2", target_bir_lowering=False, debug=False)

    xT = nc.declare_dram_parameter("xT", (D, L), BF16, isOutput=False)
    wqT = nc.declare_dram_parameter("wqT", (D, E), BF16, isOutput=False)
    wkT = nc.declare_dram_parameter("wkT", (D, E), BF16, isOutput=False)
    wvT = nc.declare_dram_parameter("wvT", (D, EA), BF16, isOutput=False)
    woT = nc.declare_dram_parameter("woT", (E, D), BF16, isOutput=False)
    bq = nc.declare_dram_parameter("bq", (MT_E, P), F32, isOutput=False)
    bk = nc.declare_dram_parameter("bk", (MT_E, P), F32, isOutput=False)
    bv = nc.declare_dram_parameter("bv", (P, EA), F32, isOutput=False)
    out = nc.declare_dram_parameter("out", (L, D), BF16, isOutput=True)

    with tile.TileContext(nc) as tc:
        with (
            tc.tile_pool(name="const", bufs=1) as const,
            tc.tile_pool(name="ps_sc", bufs=2, space="PSUM") as ps_sc,
            tc.tile_pool(name="ps_ctx", bufs=2, space="PSUM") as ps_ctx,
            tc.tile_pool(name="ps_proj", bufs=2, space="PSUM") as ps_proj,
            tc.tile_pool(name="ework", bufs=6) as ework,
            tc.tile_pool(name="small", bufs=6) as small,
            tc.tile_pool(name="outp", bufs=3) as outp,
        ):
            # ---- resident tensors ----
            xT_sb = const.tile([P, KT, L], BF16)
            wq_sb = const.tile([P, KT, E], BF16)
            wk_sb = const.tile([P, KT, E], BF16)
            wv_sb = const.tile([P, KT, EA], BF16)
            wo_sb = const.tile([P, MT_E, D], BF16)
            qT_sb = const.tile([P, MT_E, L], BF16)
            kT_sb = const.tile([P, MT_E, L], BF16)
            v_sb = const.tile([P, ST, NH, HD + 1], BF16)
            ctxT_sb = const.tile([P, MT_E, L], BF16)
            bq_sb = const.tile([P, MT_E], F32)
            bk_sb = const.tile([P, MT_E], F32)
            bv_bc = const.tile([P, EA], F32)
            act_warm = const.tile([1, 2], F32)

            # Warm the exp ACT table during the DMA shadow.
            nc.scalar.activation(act_warm[:, 1:2], act_warm[:, 0:1], AF.Exp)

            # ---- DMA prologue: spread issues over 4 engine queues ----
            def dma_x(eng, nb):
                for o in range(KT):
                    eng.dma_start(
                        xT_sb[:, o, nb * LB:(nb + 1) * LB],
                        xT[o * P:(o + 1) * P, nb * LB:(nb + 1) * LB],
                    )

            def dma_w(eng, dst, src):
                for o in range(KT):
                    eng.dma_start(dst[:, o, :], src[o * P:(o + 1) * P, :])

            # sync queue: wk (kT m0 gate), wv lower half, bv, x n1/n3
            dma_w(nc.sync, wk_sb, wkT)
            for o in range(4):
                nc.sync.dma_start(wv_sb[:, o, :], wvT[o * P:(o + 1) * P, :])
            nc.sync.dma_start(bv_bc[:, :], bv[:, :])
            nc.sync.dma_start(bq_sb[:, :], bq.rearrange("o p -> p o"))
            nc.sync.dma_start(bk_sb[:, :], bk.rearrange("o p -> p o"))
            dma_x(nc.sync, 1)
            dma_x(nc.sync, 3)
            # gpsimd queue: x n0 (kT m0 + v_0 gate), wv upper half, x n2, wo
            dma_x(nc.gpsimd, 0)
            for o in range(4, KT):
                nc.gpsimd.dma_start(wv_sb[:, o, :], wvT[o * P:(o + 1) * P, :])
            dma_x(nc.gpsimd, 2)
            for o in range(MT_E):
                nc.gpsimd.dma_start(wo_sb[:, o, :], woT[o * P:(o + 1) * P, :])
            # scalar queue: wq (after the table-warm activation)
            dma_w(nc.scalar, wq_sb, wqT)

            # ---- unit emitters ----
            def kq_halves(w_sb, dst, b_sb, m, nb):
                """k/q projection of one 512-col l-block, as two 4-MM halves."""
                cell = {}

                def h1():
                    cell["ps"] = ps_proj.tile([P, 512], F32, tag="proj",
                                              name="proj")
                    for kk in range(4):
                        nc.tensor.matmul(
                            cell["ps"][:],
                            lhsT=w_sb[:, kk, m * P:(m + 1) * P],
                            rhs=xT_sb[:, kk, nb * 512:(nb + 1) * 512],
                            start=(kk == 0),
                            stop=False,
                        )

                def h2():
                    for kk in range(4, KT):
                        nc.tensor.matmul(
                            cell["ps"][:],
                            lhsT=w_sb[:, kk, m * P:(m + 1) * P],
                            rhs=xT_sb[:, kk, nb * 512:(nb + 1) * 512],
                            start=False,
                            stop=(kk == KT - 1),
                        )
                    nc.vector.tensor_scalar_add(
                        dst[:, m, nb * 512:(nb + 1) * 512], cell["ps"][:],
                        b_sb[:, m:m + 1],
                    )

                return [h1, h2]

            def v_unit(st):
                def f():
                    ps = ps_proj.tile([P, 512], F32, tag="proj", name="proj")
                    for kk in range(KT):
                        nc.tensor.matmul(
                            ps[:, :EA],
                            lhsT=xT_sb[:, kk, st * P:(st + 1) * P],
                            rhs=wv_sb[:, kk, :],
                            start=(kk == 0),
                            stop=(kk == KT - 1),
                        )
                    nc.vector.tensor_tensor(
                        v_sb[:, st, :, :],
                        ps[:, :EA].rearrange("p (h e) -> p h e", h=NH),
                        bv_bc[:].rearrange("p (h e) -> p h e", h=NH),
                        ALU.add,
                    )

                return f

            def outproj_unit(lt, dc, alt=False):
                def f():
                    pool = ps_sc if alt else ps_proj
                    tag = "sc" if alt else "proj"
                    ps = pool.tile([P, 512], F32, tag=tag, name="ops")
                    for kk in range(MT_E):
                        nc.tensor.matmul(
                            ps[:],
                            lhsT=ctxT_sb[:, kk, lt * P:(lt + 1) * P],
                            rhs=wo_sb[:, kk, dc * 512:(dc + 1) * 512],
                            start=(kk == 0),
                            stop=(kk == MT_E - 1),
                        )
                    ot = outp.tile([P, 512], BF16, tag="ot")
                    if alt:
                        nc.scalar.copy(ot[:], ps[:])
                    else:
                        nc.vector.tensor_copy(ot[:], ps[:])
                    nc.sync.dma_start(
                        out[lt * P:(lt + 1) * P, dc * 512:(dc + 1) * 512],
                        ot[:],
                    )

                return f

            # filler queue: (need_key, closure), FIFO
            fillers = []

            def pull(n=1):
                for _ in range(min(n, len(fillers))):
                    fillers.pop(0)[1]()

            def drain_needed(key):
                while any(u[0] == key for u in fillers):
                    fillers.pop(0)[1]()

            # ---- attention block: pipelined st loop ----
            def attn_block(lb, hp, inserts=None, first_pull_slot=1):
                ctx_tiles = [
                    ps_ctx.tile([HD + 1, LB], F32, tag="ctxps", name="ctxps")
                    for _ in range(2)
                ]

                def emit_sc(st):
                    sc = ps_sc.tile([P, 2 * LB], F32, tag="sc", name="sc")
                    for hh in range(2):
                        off = hh * HD
                        nc.tensor.matmul(
                            sc[:, hh * LB:(hh + 1) * LB],
                            lhsT=kT_sb[off:off + HD, hp, st * P:(st + 1) * P],
                            rhs=qT_sb[off:off + HD, hp, lb * LB:(lb + 1) * LB],
                            start=True,
                            stop=True,
                        )
                    e_t = ework.tile([P, 2 * LB], BF16, tag="etile",
                                     name="etile")
                    nc.scalar.activation(e_t[:], sc[:], AF.Exp, scale=SCALE)
                    return e_t

                def emit_ctx(st, e_t):
                    for hh in range(2):
                        h = 2 * hp + hh
                        nc.tensor.matmul(
                            ctx_tiles[hh][:],
                            lhsT=v_sb[:, st, h, :],
                            rhs=e_t[:, hh * LB:(hh + 1) * LB],
                            start=(st == 0),
                            stop=(st == ST - 1),
                        )

                def filler_slot(st):
                    if inserts is not None:
                        for f in inserts.get(st, ()):
                            f()
                    elif st >= first_pull_slot:
                        pull(1)

                e_prev = emit_sc(0)
                for st in range(1, ST):
                    e_cur = emit_sc(st)
                    filler_slot(st)
                    emit_ctx(st - 1, e_prev)
                    e_prev = e_cur
                filler_slot(ST)
                emit_ctx(ST - 1, e_prev)

                # normalisation: copy ctx out of PSUM at once (frees the
                # accumulation banks), one batched reciprocal, then scale.
                cc = [small.tile([HD, LB], F32, tag="cc", name="cc")
                      for _ in range(2)]
                rs = small.tile([HD, LB], F32, tag="rs", name="rs")
                nc.vector.memset(rs[:33, :], 1.0)
                for hh in range(2):
                    nc.vector.tensor_copy(cc[hh][:], ctx_tiles[hh][:HD, :])
                    nc.vector.tensor_copy(rs[32 * hh:32 * hh + 1, :],
                                          ctx_tiles[hh][HD:HD + 1, :])
                rp = small.tile([HD, LB], F32, tag="rp", name="rp")
                nc.vector.reciprocal(rp[:33, :], rs[:33, :])
                rp1 = small.tile([1, LB], F32, tag="rp1", name="rp1")
                nc.vector.tensor_copy(rp1[:], rp[32:33, :])
                for hh in range(2):
                    h = 2 * hp + hh
                    off = (h * HD) % P
                    bcast = small.tile([HD, LB], F32, tag="bcast",
                                       name="bcast")
                    nc.gpsimd.partition_broadcast(
                        bcast[:], rp[0:1, :] if hh == 0 else rp1[:])
                    nc.vector.tensor_tensor(
                        ctxT_sb[off:off + HD, hp, lb * LB:(lb + 1) * LB],
                        cc[hh][:],
                        bcast[:],
                        ALU.mult,
                    )

            # ---- ramp: bare minimum before the first score matmul ----
            for h in kq_halves(wk_sb, kT_sb, bk_sb, 0, 0):
                h()
            for h in kq_halves(wq_sb, qT_sb, bq_sb, 0, 0):
                h()

            # first block carries the k/v projections just-in-time:
            # v_{st-1} lands in slot st (just before ctx_{st-1}).
            kn = {nb: kq_halves(wk_sb, kT_sb, bk_sb, 0, nb) for nb in (1, 2, 3)}
            q1 = kq_halves(wq_sb, qT_sb, bq_sb, 0, 1)
            inserts0 = {
                1: [v_unit(0), kn[1][0]],
                2: [v_unit(1), kn[1][1]],
                3: [v_unit(2)],
                4: [v_unit(3), kn[2][0]],
                5: [v_unit(4), kn[2][1]],
                6: [v_unit(5)],
                7: [v_unit(6), kn[3][0]],
                8: [v_unit(7), kn[3][1]],
                9: [v_unit(8)],
                10: [v_unit(9), q1[0]],
                11: [v_unit(10), q1[1]],
                12: [v_unit(11)],
                13: [v_unit(12), v_unit(13)],
                14: [v_unit(14)],
                15: [v_unit(15)],
            }

            # filler queue, in global dependency order
            for h in kq_halves(wq_sb, qT_sb, bq_sb, 0, 2):
                fillers.append((("q", 0, 2), h))
            for nb in range(NLB):
                for h in kq_halves(wk_sb, kT_sb, bk_sb, 1, nb):
                    fillers.append((("k", 1), h))
            for h in kq_halves(wq_sb, qT_sb, bq_sb, 0, 3):
                fillers.append((("q", 0, 3), h))
            for lb in range(NLB):
                for h in kq_halves(wq_sb, qT_sb, bq_sb, 1, lb):
                    fillers.append((("q", 1, lb), h))

            # ---- main stream: hp=0 blocks, then hp=1 blocks ----
            attn_block(0, 0, inserts=inserts0)
            for lb in range(1, NLB):
                drain_needed(("q", 0, lb))
                attn_block(lb, 0)
            for lb in range(NLB):
                drain_needed(("k", 1))
                drain_needed(("q", 1, lb))
                attn_block(lb, 1, first_pull_slot=6)
                last = lb == NLB - 1
                for i in range(NLB):
                    lt = lb * NLB + i
                    for dc in range(D // 512):
                        fillers.append(
                            (None, outproj_unit(lt, dc, alt=last and i % 2)))
            while fillers:
                fillers.pop(0)[1]()
    nc.compile()
    return nc


def _augment_wv(wv_slice):
    """Interleave a zero column after each head's 64 value columns."""
    e, d = wv_slice.shape
    nh = e // HD
    aug = np.zeros((nh * (HD + 1), d), dtype=np.float32)
    for h in range(nh):
        aug[h * (HD + 1):h * (HD + 1) + HD] = wv_slice[h * HD:(h + 1) * HD]
    return aug


def _augment_bv(bv_slice):
    """bv with 1.0 in each head's ones-column slot."""
    e = bv_slice.shape[0]
    nh = e // HD
    aug = np.zeros(nh * (HD + 1), dtype=np.float32)
    for h in range(nh):
        aug[h * (HD + 1):h * (HD + 1) + HD] = bv_slice[h * HD:(h + 1) * HD]
        aug[h * (HD + 1) + HD] = 1.0
    return aug


def _core_in_map(core, x, Wq, bq, Wk, bk, Wv, bv, Wo):
    b = core // GROUPS
    g = core % GROUPS
    sl = slice(g * E_CORE, (g + 1) * E_CORE)
    bf = ml_dtypes.bfloat16
    return {
        "xT": np.ascontiguousarray(x[b].T.astype(bf)),
        "wqT": np.ascontiguousarray(Wq[sl, :].T.astype(bf)),
        "wkT": np.ascontiguousarray(Wk[sl, :].T.astype(bf)),
        "wvT": np.ascontiguousarray(_augment_wv(Wv[sl, :]).T.astype(bf)),
        "woT": np.ascontiguousarray(Wo[:, sl].T.astype(bf)),
        "bq": np.ascontiguousarray(bq[sl].reshape(-1, 128)),
        "bk": np.ascontiguousarray(bk[sl].reshape(-1, 128)),
        "bv": np.ascontiguousarray(
            np.tile(_augment_bv(bv[sl]).reshape(1, -1), (128, 1))),
    }


_NC_CACHE = {}


def _get_kernel(L, D, E):
    key = (L, D, E)
    if key not in _NC_CACHE:
        _NC_CACHE[key] = build_core_kernel(L=L, D=D, E=E)
    return _NC_CACHE[key]


LAST_RESULT = None


def kernel(x, Wq, bq, Wk, bk, Wv, bv, Wo, bo, trace=False, tmpdir=None):
    x = np.asarray(x, dtype=np.float32)
    Wq = np.asarray(Wq, dtype=np.float32)
    Wk = np.asarray(Wk, dtype=np.float32)
    Wv = np.asarray(Wv, dtype=np.float32)
    Wo = np.asarray(Wo, dtype=np.float32)
    bq = np.asarray(bq, dtype=np.float32)
    bk = np.asarray(bk, dtype=np.float32)
    bv = np.asarray(bv, dtype=np.float32)
    bo = np.asarray(bo, dtype=np.float32)

    Bx, L, D = x.shape
    nc = _get_kernel(L, D, E_CORE)

    in_maps = [
        _core_in_map(core, x, Wq, bq, Wk, bk, Wv, bv, Wo)
        for core in range(N_CORES)
    ]

    global LAST_RESULT
    LAST_RESULT = run_bass_kernel_spmd(
        nc, in_maps, core_ids=list(range(N_CORES)), trace=trace, tmpdir=tmpdir,
    )
    outs = [
        np.asarray(LAST_RESULT.results[c]["out"]).astype(np.float32)
        for c in range(N_CORES)
    ]
    full = np.stack(
        [sum(outs[b * GROUPS:(b + 1) * GROUPS]) for b in range(Bx)], axis=0
    )
    return (full + bo).astype(np.float32)


# revision 29
# speedup vs baseline: 1.4061x; 1.0067x over previous
"""Multi-head attention layer on 8 Trainium2 NeuronCores.

Sharding: 8 cores = 2 (batch) x 4 (head groups of 4 heads).  Each core
computes its batch's attention for its 4 heads plus the partial output
projection (row-parallel); the host sums the 4 partials per batch.

Schedule (single software-pipelined stream per core):
  The softmax exp on ScalarE (ACTIVATE, ~142us total over 128 calls) is
  the hard bottleneck, so the program keeps ScalarE saturated from ~12us
  on.  Attention blocks run hp=0 for all four l-blocks first, then hp=1,
  with the first block's st-loop carrying the remaining k/v projections
  as just-in-time inserts.  All other projection work and the output
  projection are chopped into small "filler" units pulled into the PE
  stream between score groups.  Input DMA issues are spread across the
  sync/gpsimd/scalar queues so descriptor issue rate doesn't gate the
  ramp.  The softmax normalisation copies ctx out of PSUM immediately
  (freeing the accumulation banks for the next block) and batches the
  two per-head reciprocals into one DVE instruction.

Per-core math (PE matmuls in bf16, fp32 PSUM accumulate):
  kT/qT = W_g @ x^T           (e=256 partitions, L free)
  v  = x @ Wv_g^T             (s x e layout, + per-head ones column)
  per head pair, l-block: scoresT = k_h q_h^T (row-tiled concurrent),
    E = exp(scale*scoresT) on ScalarE, ctx_aug += v_aug^T E (row 64 =
    rowsum), ctxT = ctx * (1/rowsum)
  out_partial = ctxT^T @ Wo_g^T, written bf16; host sums 4 partials.
"""

import ml_dtypes
import numpy as np

import concourse.bass as bass
import concourse.mybir as mybir
import concourse.tile as tile
from concourse import bacc
from concourse.bass_utils import run_bass_kernel_spmd

F32 = mybir.dt.float32
BF16 = mybir.dt.bfloat16
AF = mybir.ActivationFunctionType
ALU = mybir.AluOpType

P = 128
HD = 64  # head dim

D_MODEL = 1024
N_HEADS = 16
B = 2
L_FULL = 2048
N_CORES = 8
GROUPS = 4  # head groups (tensor parallel)
E_CORE = D_MODEL // GROUPS  # 256 output dims per core for q/k/v


def build_core_kernel(L=2048, D=1024, E=256, LB=512):
    """One core: x[b] attention over E//64 heads. Returns compiled Bacc."""
    assert L % LB == 0 and LB % P == 0 and D % P == 0 and E % HD == 0
    KT = D // P          # contraction tiles over d_model
    MT_E = E // P        # e tiles (q/k partition tiles)
    NLB = L // LB        # l blocks
    ST = L // P          # s tiles
    NH = E // HD         # heads on this core
    HP = NH // 2         # head pairs
    EA = NH * (HD + 1)   # v columns incl. per-head ones column
    SCALE = HD ** -0.5
    assert LB == 512 and MT_E == HP and NLB == 4 and ST == 16

    nc = bacc.Bacc("TRN2", target_bir_lowering=False, debug=False)

    # x is passed pre-packed on the host so each [P, LB] chunk (nb, o) is
    # a fully contiguous 128KB block; likewise the output is written as
    # contiguous [P, 512] chunks and re-ordered on the host.
    xT = nc.declare_dram_parameter("xT", ((L // LB) * (D // P) * P, LB),
                                   BF16, isOutput=False)
    wqT = nc.declare_dram_parameter("wqT", (D, E), BF16, isOutput=False)
    wkT = nc.declare_dram_parameter("wkT", (D, E), BF16, isOutput=False)
    wvT = nc.declare_dram_parameter("wvT", (D, EA), BF16, isOutput=False)
    woT = nc.declare_dram_parameter("woT", (E, D), BF16, isOutput=False)
    bq = nc.declare_dram_parameter("bq", (MT_E, P), F32, isOutput=False)
    bk = nc.declare_dram_parameter("bk", (MT_E, P), F32, isOutput=False)
    bv = nc.declare_dram_parameter("bv", (P, EA), F32, isOutput=False)
    out = nc.declare_dram_parameter("out", ((L // P) * (D // 512) * P, 512),
                                    BF16, isOutput=True)

    with tile.TileContext(nc) as tc:
        with (
            tc.tile_pool(name="const", bufs=1) as const,
            tc.tile_pool(name="ps_sc", bufs=2, space="PSUM") as ps_sc,
            tc.tile_pool(name="ps_ctx", bufs=2, space="PSUM") as ps_ctx,
            tc.tile_pool(name="ps_proj", bufs=2, space="PSUM") as ps_proj,
            tc.tile_pool(name="ework", bufs=6) as ework,
            tc.tile_pool(name="small", bufs=6) as small,
            tc.tile_pool(name="outp", bufs=3) as outp,
        ):
            # ---- resident tensors ----
            xT_sb = const.tile([P, KT, L], BF16)
            wq_sb = const.tile([P, KT, E], BF16)
            wk_sb = const.tile([P, KT, E], BF16)
            wv_sb = const.tile([P, KT, EA], BF16)
            wo_sb = const.tile([P, MT_E, D], BF16)
            qT_sb = const.tile([P, MT_E, L], BF16)
            kT_sb = const.tile([P, MT_E, L], BF16)
            v_sb = const.tile([P, ST, NH, HD + 1], BF16)
            ctxT_sb = const.tile([P, MT_E, L], BF16)
            bq_sb = const.tile([P, MT_E], F32)
            bk_sb = const.tile([P, MT_E], F32)
            bv_bc = const.tile([P, EA], F32)
            act_warm = const.tile([1, 2], F32)
            rs = const.tile([HD, LB], F32)

            # Warm the exp ACT table during the DMA shadow; zero the rowsum
            # staging tile once (its rows 1..31 feed the batched reciprocal).
            nc.scalar.activation(act_warm[:, 1:2], act_warm[:, 0:1], AF.Exp)
            nc.vector.memset(rs[:33, :], 1.0)

            # ---- DMA prologue: spread issues over 4 engine queues ----
            def dma_x(eng, nb):
                for o in range(KT):
                    r = (nb * KT + o) * P
                    eng.dma_start(
                        xT_sb[:, o, nb * LB:(nb + 1) * LB],
                        xT[r:r + P, :],
                    )

            def dma_w(eng, dst, src):
                for o in range(KT):
                    eng.dma_start(dst[:, o, :], src[o * P:(o + 1) * P, :])

            # sync queue: wk (kT m0 gate), wv lower half, bv, x n1/n3
            dma_w(nc.sync, wk_sb, wkT)
            for o in range(4):
                nc.sync.dma_start(wv_sb[:, o, :], wvT[o * P:(o + 1) * P, :])
            nc.sync.dma_start(bv_bc[:, :], bv[:, :])
            nc.sync.dma_start(bq_sb[:, :], bq.rearrange("o p -> p o"))
            nc.sync.dma_start(bk_sb[:, :], bk.rearrange("o p -> p o"))
            dma_x(nc.sync, 1)
            dma_x(nc.sync, 3)
            # gpsimd queue: x n0 (kT m0 + v_0 gate), wv upper half, x n2, wo
            dma_x(nc.gpsimd, 0)
            for o in range(4, KT):
                nc.gpsimd.dma_start(wv_sb[:, o, :], wvT[o * P:(o + 1) * P, :])
            dma_x(nc.gpsimd, 2)
            for o in range(MT_E):
                nc.gpsimd.dma_start(wo_sb[:, o, :], woT[o * P:(o + 1) * P, :])
            # scalar queue: wq (after the table-warm activation)
            dma_w(nc.scalar, wq_sb, wqT)

            # ---- unit emitters ----
            def kq_halves(w_sb, dst, b_sb, m, nb):
                """k/q projection of one 512-col l-block, as two 4-MM halves."""
                cell = {}

                def h1():
                    cell["ps"] = ps_proj.tile([P, 512], F32, tag="proj",
                                              name="proj")
                    for kk in range(4):
                        nc.tensor.matmul(
                            cell["ps"][:],
                            lhsT=w_sb[:, kk, m * P:(m + 1) * P],
                            rhs=xT_sb[:, kk, nb * 512:(nb + 1) * 512],
                            start=(kk == 0),
                            stop=False,
                        )

                def h2():
                    for kk in range(4, KT):
                        nc.tensor.matmul(
                            cell["ps"][:],
                            lhsT=w_sb[:, kk, m * P:(m + 1) * P],
                            rhs=xT_sb[:, kk, nb * 512:(nb + 1) * 512],
                            start=False,
                            stop=(kk == KT - 1),
                        )
                    nc.vector.tensor_scalar_add(
                        dst[:, m, nb * 512:(nb + 1) * 512], cell["ps"][:],
                        b_sb[:, m:m + 1],
                    )

                return [h1, h2]

            def v_unit(st):
                def f():
                    ps = ps_proj.tile([P, 512], F32, tag="proj", name="proj")
                    for kk in range(KT):
                        nc.tensor.matmul(
                            ps[:, :EA],
                            lhsT=xT_sb[:, kk, st * P:(st + 1) * P],
                            rhs=wv_sb[:, kk, :],
                            start=(kk == 0),
                            stop=(kk == KT - 1),
                        )
                    nc.vector.tensor_tensor(
                        v_sb[:, st, :, :],
                        ps[:, :EA].rearrange("p (h e) -> p h e", h=NH),
                        bv_bc[:].rearrange("p (h e) -> p h e", h=NH),
                        ALU.add,
                    )

                return f

            def outproj_unit(lt, dc, alt=False):
                def f():
                    pool = ps_sc if alt else ps_proj
                    tag = "sc" if alt else "proj"
                    ps = pool.tile([P, 512], F32, tag=tag, name="ops")
                    for kk in range(MT_E):
                        nc.tensor.matmul(
                            ps[:],
                            lhsT=ctxT_sb[:, kk, lt * P:(lt + 1) * P],
                            rhs=wo_sb[:, kk, dc * 512:(dc + 1) * 512],
                            start=(kk == 0),
                            stop=(kk == MT_E - 1),
                        )
                    ot = outp.tile([P, 512], BF16, tag="ot")
                    if alt:
                        nc.scalar.copy(ot[:], ps[:])
                    else:
                        nc.vector.tensor_copy(ot[:], ps[:])
                    r = (lt * (D // 512) + dc) * P
                    nc.sync.dma_start(out[r:r + P, :], ot[:])

                return f

            # filler queue: (need_key, closure), FIFO
            fillers = []

            def pull(n=1):
                for _ in range(min(n, len(fillers))):
                    fillers.pop(0)[1]()

            def drain_needed(key):
                while any(u[0] == key for u in fillers):
                    fillers.pop(0)[1]()

            # ---- attention block: pipelined st loop ----
            def attn_block(lb, hp, inserts=None, first_pull_slot=1):
                ctx_tiles = [
                    ps_ctx.tile([HD + 1, LB], F32, tag="ctxps", name="ctxps")
                    for _ in range(2)
                ]

                def emit_sc(st):
                    sc = ps_sc.tile([P, 2 * LB], F32, tag="sc", name="sc")
                    for hh in range(2):
                        off = hh * HD
                        nc.tensor.matmul(
                            sc[:, hh * LB:(hh + 1) * LB],
                            lhsT=kT_sb[off:off + HD, hp, st * P:(st + 1) * P],
                            rhs=qT_sb[off:off + HD, hp, lb * LB:(lb + 1) * LB],
                            start=True,
                            stop=True,
                        )
                    e_t = ework.tile([P, 2 * LB], BF16, tag="etile",
                                     name="etile")
                    nc.scalar.activation(e_t[:], sc[:], AF.Exp, scale=SCALE)
                    return e_t

                def emit_ctx(st, e_t):
                    for hh in range(2):
                        h = 2 * hp + hh
                        nc.tensor.matmul(
                            ctx_tiles[hh][:],
                            lhsT=v_sb[:, st, h, :],
                            rhs=e_t[:, hh * LB:(hh + 1) * LB],
                            start=(st == 0),
                            stop=(st == ST - 1),
                        )

                def filler_slot(st):
                    if inserts is not None:
                        for f in inserts.get(st, ()):
                            f()
                    elif st >= first_pull_slot:
                        pull(1)

                e_prev = emit_sc(0)
                for st in range(1, ST):
                    e_cur = emit_sc(st)
                    filler_slot(st)
                    emit_ctx(st - 1, e_prev)
                    e_prev = e_cur
                filler_slot(ST)
                emit_ctx(ST - 1, e_prev)

                # normalisation: copy ctx out of PSUM at once (frees the
                # accumulation banks), one batched reciprocal, then scale.
                cc = [small.tile([HD, LB], F32, tag="cc", name="cc")
                      for _ in range(2)]
                for hh in range(2):
                    nc.vector.tensor_copy(cc[hh][:], ctx_tiles[hh][:HD, :])
                    nc.vector.tensor_copy(rs[32 * hh:32 * hh + 1, :],
                                          ctx_tiles[hh][HD:HD + 1, :])
                rp = small.tile([HD, LB], F32, tag="rp", name="rp")
                nc.vector.reciprocal(rp[:33, :], rs[:33, :])
                rp1 = small.tile([1, LB], F32, tag="rp1", name="rp1")
                nc.vector.tensor_copy(rp1[:], rp[32:33, :])
                for hh in range(2):
                    h = 2 * hp + hh
                    off = (h * HD) % P
                    bcast = small.tile([HD, LB], F32, tag="bcast",
                                       name="bcast")
                    nc.gpsimd.partition_broadcast(
                        bcast[:], rp[0:1, :] if hh == 0 else rp1[:])
                    nc.vector.tensor_tensor(
                        ctxT_sb[off:off + HD, hp, lb * LB:(lb + 1) * LB],
                        cc[hh][:],
                        bcast[:],
                        ALU.mult,
                    )

            # ---- ramp: bare minimum before the first score matmul ----
            for h in kq_halves(wk_sb, kT_sb, bk_sb, 0, 0):
                h()
            for h in kq_halves(wq_sb, qT_sb, bq_sb, 0, 0):
                h()

            # first block carries the k/v projections just-in-time:
            # v_{st-1} lands in slot st (just before ctx_{st-1}).
            kn = {nb: kq_halves(wk_sb, kT_sb, bk_sb, 0, nb) for nb in (1, 2, 3)}
            q1 = kq_halves(wq_sb, qT_sb, bq_sb, 0, 1)
            inserts0 = {
                1: [v_unit(0), kn[1][0]],
                2: [v_unit(1), kn[1][1]],
                3: [v_unit(2)],
                4: [v_unit(3), kn[2][0]],
                5: [v_unit(4), kn[2][1]],
                6: [v_unit(5)],
                7: [v_unit(6), kn[3][0]],
                8: [v_unit(7), kn[3][1]],
                9: [v_unit(8)],
                10: [v_unit(9), q1[0]],
                11: [v_unit(10), q1[1]],
                12: [v_unit(11)],
                13: [v_unit(12), v_unit(13)],
                14: [v_unit(14)],
                15: [v_unit(15)],
            }

            # filler queue, in global dependency order
            for h in kq_halves(wq_sb, qT_sb, bq_sb, 0, 2):
                fillers.append((("q", 0, 2), h))
            for nb in range(NLB):
                for h in kq_halves(wk_sb, kT_sb, bk_sb, 1, nb):
                    fillers.append((("k", 1), h))
            for h in kq_halves(wq_sb, qT_sb, bq_sb, 0, 3):
                fillers.append((("q", 0, 3), h))
            for lb in range(NLB):
                for h in kq_halves(wq_sb, qT_sb, bq_sb, 1, lb):
                    fillers.append((("q", 1, lb), h))

            # ---- main stream: hp=0 blocks, then hp=1 blocks ----
            attn_block(0, 0, inserts=inserts0)
            for lb in range(1, NLB):
                drain_needed(("q", 0, lb))
                attn_block(lb, 0)
            for lb in range(NLB):
                drain_needed(("k", 1))
                drain_needed(("q", 1, lb))
                attn_block(lb, 1, first_pull_slot=10)
                last = lb == NLB - 1
                for i in range(NLB):
                    lt = lb * NLB + i
                    for dc in range(D // 512):
                        fillers.append(
                            (None, outproj_unit(lt, dc, alt=last and i % 2)))
            while fillers:
                fillers.pop(0)[1]()
    nc.compile()
    return nc


def _augment_wv(wv_slice):
    """Interleave a zero column after each head's 64 value columns."""
    e, d = wv_slice.shape
    nh = e // HD
    aug = np.zeros((nh * (HD + 1), d), dtype=np.float32)
    for h in range(nh):
        aug[h * (HD + 1):h * (HD + 1) + HD] = wv_slice[h * HD:(h + 1) * HD]
    return aug


def _augment_bv(bv_slice):
    """bv with 1.0 in each head's ones-column slot."""
    e = bv_slice.shape[0]
    nh = e // HD
    aug = np.zeros(nh * (HD + 1), dtype=np.float32)
    for h in range(nh):
        aug[h * (HD + 1):h * (HD + 1) + HD] = bv_slice[h * HD:(h + 1) * HD]
        aug[h * (HD + 1) + HD] = 1.0
    return aug


def _core_in_map(core, x, Wq, bq, Wk, bk, Wv, bv, Wo):
    b = core // GROUPS
    g = core % GROUPS
    sl = slice(g * E_CORE, (g + 1) * E_CORE)
    bf = ml_dtypes.bfloat16
    D, L = x.shape[2], x.shape[1]
    KT, NLB = D // 128, L // 512
    xT_pack = (x[b].T.astype(bf)
               .reshape(KT, 128, NLB, 512)
               .transpose(2, 0, 1, 3)
               .reshape(NLB * KT * 128, 512))
    return {
        "xT": np.ascontiguousarray(xT_pack),
        "wqT": np.ascontiguousarray(Wq[sl, :].T.astype(bf)),
        "wkT": np.ascontiguousarray(Wk[sl, :].T.astype(bf)),
        "wvT": np.ascontiguousarray(_augment_wv(Wv[sl, :]).T.astype(bf)),
        "woT": np.ascontiguousarray(Wo[:, sl].T.astype(bf)),
        "bq": np.ascontiguousarray(bq[sl].reshape(-1, 128)),
        "bk": np.ascontiguousarray(bk[sl].reshape(-1, 128)),
        "bv": np.ascontiguousarray(
            np.tile(_augment_bv(bv[sl]).reshape(1, -1), (128, 1))),
    }


_NC_CACHE = {}


def _get_kernel(L, D, E):
    key = (L, D, E)
    if key not in _NC_CACHE:
        _NC_CACHE[key] = build_core_kernel(L=L, D=D, E=E)
    return _NC_CACHE[key]


LAST_RESULT = None


def kernel(x, Wq, bq, Wk, bk, Wv, bv, Wo, bo, trace=False, tmpdir=None):
    x = np.asarray(x, dtype=np.float32)
    Wq = np.asarray(Wq, dtype=np.float32)
    Wk = np.asarray(Wk, dtype=np.float32)
    Wv = np.asarray(Wv, dtype=np.float32)
    Wo = np.asarray(Wo, dtype=np.float32)
    bq = np.asarray(bq, dtype=np.float32)
    bk = np.asarray(bk, dtype=np.float32)
    bv = np.asarray(bv, dtype=np.float32)
    bo = np.asarray(bo, dtype=np.float32)

    Bx, L, D = x.shape
    nc = _get_kernel(L, D, E_CORE)

    in_maps = [
        _core_in_map(core, x, Wq, bq, Wk, bk, Wv, bv, Wo)
        for core in range(N_CORES)
    ]

    global LAST_RESULT
    LAST_RESULT = run_bass_kernel_spmd(
        nc, in_maps, core_ids=list(range(N_CORES)), trace=trace, tmpdir=tmpdir,
    )
    outs = [
        np.asarray(LAST_RESULT.results[c]["out"]).astype(np.float32)
        .reshape(L // 128, D // 512, 128, 512)
        .transpose(0, 2, 1, 3)
        .reshape(L, D)
        for c in range(N_CORES)
    ]
    full = np.stack(
        [sum(outs[b * GROUPS:(b + 1) * GROUPS]) for b in range(Bx)], axis=0
    )
    return (full + bo).astype(np.float32)
